# revision 3
# baseline (speedup 1.0000x reference)
"""Trainium2 Bass kernel for KANPolyLayer:
    y[b,o] = sum_{i,p} x[b,i]^p * coeffs[o,i,p] + bias[o],  p = 0..4

Math: y = sum_{p=1..4} (x^p) @ C_p^T + (bias + colsum(C_0)), with
C_p = coeffs[:, :, p].  Implemented as 4 accumulated GEMM planes in
float32r (FP22 truncated fp32, full PE rate) with powers computed
on-chip.  The p=0 constant row and bias are accumulated on-device with
tiny matmuls and broadcast into the output during the PSUM->SBUF copy.

Sharding (8 cores): 4 batch groups x 2 out-dim groups.
  core c -> (bg, og) = (c // 2, c % 2)
  per-core x slice:    rows [bg*1024, (bg+1)*1024)   (transposed on host)
  per-core out slice:  cols [og*512, (og+1)*512)
Each core computes a disjoint (1024 x 512) block of y; host concatenates.
"""

from contextlib import ExitStack

import numpy as np

import concourse.bacc as bacc
import concourse.bass as bass
import concourse.mybir as mybir
import concourse.tile as tile
from concourse.bass_utils import run_bass_kernel_spmd

F32 = mybir.dt.float32
F32R = mybir.dt.float32r

B, I, O = 4096, 1024, 1024  # batch, in_dim, out_dim
BW, OW = 4, 2               # batch groups x out-dim groups (8 cores)
BS, OS = B // BW, O // OW   # per-core batch (1024) and out (512)
NK = I // 128               # contraction tiles (8)
NQ = 4                      # batch processed in quarters of 256 cols
QB = BS // NQ               # 256
NJ = QB // 128              # output row-tiles per quarter (2)

_CACHE: dict = {}


def _build():
    nc = bacc.Bacc("TRN2", target_bir_lowering=False, debug=False, num_devices=8)

    xt = nc.dram_tensor("xt", [I, BS], F32, kind="ExternalInput")      # [i, b]
    ct = nc.dram_tensor("ct", [5, I, OS], F32, kind="ExternalInput")   # [p, i, o]
    bias = nc.dram_tensor("bias", [1, OS], F32, kind="ExternalInput")
    y = nc.dram_tensor("y", [BS, OS], F32, kind="ExternalOutput")      # [b, o]

    with tile.TileContext(nc) as tc, ExitStack() as ctx:
        cons = ctx.enter_context(tc.tile_pool(name="cons", bufs=1))
        cpool = ctx.enter_context(tc.tile_pool(name="coef", bufs=1))
        c0pool = ctx.enter_context(tc.tile_pool(name="c0", bufs=1))
        xpool = ctx.enter_context(tc.tile_pool(name="xq", bufs=2))
        ppool = ctx.enter_context(tc.tile_pool(name="pow", bufs=2))
        opool = ctx.enter_context(tc.tile_pool(name="out", bufs=3))
        pspool = ctx.enter_context(
            tc.tile_pool(name="ps", bufs=4, space=bass.MemorySpace.PSUM)
        )
        pssm = ctx.enter_context(
            tc.tile_pool(name="pssm", bufs=2, space=bass.MemorySpace.PSUM)
        )

        ones_f32 = cons.tile([128, 128], F32)
        nc.vector.memset(ones_f32[:], 1.0)
        ones = cons.tile([128, 128], F32R)
        nc.vector.tensor_copy(ones[:], ones_f32[:])
        bias_sb = cons.tile([1, OS], F32R)
        nc.sync.dma_start(bias_sb[:], bias[:].bitcast(F32R))

        # biasrow[o] = bias[o] + sum_i C0[i, o], accumulated on PE
        ps_b = pssm.tile([1, OS], F32)
        for k in range(NK):
            c0t = c0pool.tile([128, OS], F32R, tag=f"c0_{k}")
            nc.sync.dma_start(c0t[:], ct[0, k * 128:(k + 1) * 128, :].bitcast(F32R))
            nc.tensor.matmul(
                ps_b[:],
                ones[:, 0:1],
                c0t[:],
                start=(k == 0),
                stop=False,
            )
        nc.tensor.matmul(
            ps_b[:],
            ones[0:1, 0:1],
            bias_sb[:],
            start=False,
            stop=True,
        )
        biasrow = cons.tile([1, OS], F32R)
        nc.vector.tensor_copy(biasrow[:], ps_b[:])

        # replicate biasrow across all 128 partitions
        ps_r = pssm.tile([128, OS], F32)
        nc.tensor.matmul(ps_r[:], ones[0:1, :], biasrow[:])
        biasrep = cons.tile([128, OS], F32)
        nc.vector.tensor_copy(biasrep[:], ps_r[:])

        # resident coefficient planes p=1..4: tiles [i=128, o=OS]
        cp = {}
        for k in range(NK):
            for p in range(1, 5):
                t = cpool.tile([128, OS], F32R, tag=f"cp_{p}_{k}")
                nc.sync.dma_start(
                    t[:], ct[p, k * 128:(k + 1) * 128, :].bitcast(F32R)
                )
                cp[(p, k)] = t

        for q in range(NQ):
            pow_ = {}
            for k in range(NK):
                xq = xpool.tile([128, QB], F32R, tag=f"x_{k}")
                nc.sync.dma_start(
                    xq[:],
                    xt[k * 128:(k + 1) * 128, q * QB:(q + 1) * QB].bitcast(F32R),
                )
                p2 = ppool.tile([128, QB], F32R, tag=f"p2_{k}")
                p3 = ppool.tile([128, QB], F32R, tag=f"p3_{k}")
                p4 = ppool.tile([128, QB], F32R, tag=f"p4_{k}")
                nc.scalar.square(p2[:], xq[:])
                nc.vector.tensor_mul(p3[:], p2[:], xq[:])
                nc.vector.tensor_mul(p4[:], p2[:], p2[:])
                pow_[(1, k)] = xq
                pow_[(2, k)] = p2
                pow_[(3, k)] = p3
                pow_[(4, k)] = p4

            for jj in range(NJ):
                ps = pspool.tile([128, OS], F32)
                n = 0
                for k in range(NK):
                    for p in range(1, 5):
                        nc.tensor.matmul(
                            ps[:],
                            pow_[(p, k)][:, jj * 128:(jj + 1) * 128],
                            cp[(p, k)][:],
                            start=(n == 0),
                            stop=(n == 4 * NK - 1),
                        )
                        n += 1
                ot = opool.tile([128, OS], F32)
                nc.vector.tensor_add(ot[:], ps[:], biasrep[:])
                j = q * NJ + jj
                nc.sync.dma_start(y[j * 128:(j + 1) * 128, :], ot[:])

    nc.compile()
    return nc


def _get_nc():
    if "nc" not in _CACHE:
        _CACHE["nc"] = _build()
    return _CACHE["nc"]


def _make_in_maps(x, coeffs, bias):
    x = np.asarray(x, dtype=np.float32)
    coeffs = np.asarray(coeffs, dtype=np.float32)
    bias = np.asarray(bias, dtype=np.float32)

    xts = [
        np.ascontiguousarray(x[bg * BS:(bg + 1) * BS, :].T) for bg in range(BW)
    ]
    cts = [
        np.ascontiguousarray(
            coeffs[og * OS:(og + 1) * OS, :, :].transpose(2, 1, 0)
        )
        for og in range(OW)
    ]
    in_maps = []
    for c in range(BW * OW):
        bg, og = c // OW, c % OW
        in_maps.append(
            {
                "xt": xts[bg],
                "ct": cts[og],
                "bias": np.ascontiguousarray(bias[:, og * OS:(og + 1) * OS]),
            }
        )
    return in_maps


def _gather(results):
    y = np.empty((B, O), dtype=np.float32)
    for c, res in enumerate(results):
        bg, og = c // OW, c % OW
        y[bg * BS:(bg + 1) * BS, og * OS:(og + 1) * OS] = res["y"]
    return y


def run(x, coeffs, bias, trace=False, **trace_kwargs):
    nc = _get_nc()
    in_maps = _make_in_maps(x, coeffs, bias)
    br = run_bass_kernel_spmd(
        nc, in_maps, list(range(BW * OW)), trace=trace, **trace_kwargs
    )
    return _gather(br.results), br


def kernel(x, coeffs, bias):
    out, _ = run(x, coeffs, bias)
    return out


# revision 6
# speedup vs baseline: 1.1968x; 1.1968x over previous
"""Trainium2 Bass kernel for KANPolyLayer:
    y[b,o] = sum_{i,p} x[b,i]^p * coeffs[o,i,p] + bias[o],  p = 0..4

Math: y = sum_{p=1..4} (x^p) @ C_p^T + (bias + colsum(C_0)), with
C_p = coeffs[:, :, p].  Implemented as 4 accumulated GEMM planes in
float32r (FP22 truncated fp32, full PE rate) with powers computed
on-chip (ScalarE square + VectorE muls).

Per-core schedule: the x^p power slabs ([i, b] layout) are resident in
SBUF; coefficient tiles stream through a small ring.  All 8 output
groups (4 o-tiles x 2 b-halves) accumulate concurrently in 8 PSUM
banks, so each arriving coefficient tile immediately unlocks 8 matmuls
and the PE never waits on the 10 MB coefficient stream.  The p=0
constant column and bias are reduced on-device with small matmuls into
a PSUM column, then applied as a per-partition scalar during the
PSUM->SBUF copy.  The kernel computes yT = [o, b]; host transposes.

Sharding (8 cores): 4 batch groups x 2 out-dim groups.
  core c -> (bg, og) = (c // 2, c % 2)
  per-core x slice:    rows [bg*1024, (bg+1)*1024)   (transposed on host)
  per-core out slice:  cols [og*512, (og+1)*512)
Each core computes a disjoint (512 x 1024) block of yT; host gathers.
"""

from contextlib import ExitStack

import numpy as np

import concourse.bacc as bacc
import concourse.bass as bass
import concourse.mybir as mybir
import concourse.tile as tile
from concourse.bass_utils import run_bass_kernel_spmd

F32 = mybir.dt.float32
F32R = mybir.dt.float32r

B, I, O = 4096, 1024, 1024  # batch, in_dim, out_dim
BW, OW = 4, 2               # batch groups x out-dim groups (8 cores)
BS, OS = B // BW, O // OW   # per-core batch (1024) and out (512)
NK = I // 128               # contraction tiles (8)
NT = OS // 128              # o-tiles (4)
NH = BS // 512              # b-halves (2)

_CACHE: dict = {}


def _build():
    nc = bacc.Bacc("TRN2", target_bir_lowering=False, debug=False, num_devices=8)

    xt = nc.dram_tensor("xt", [I, BS], F32, kind="ExternalInput")      # [i, b]
    ct = nc.dram_tensor("ct", [4, I, OS], F32, kind="ExternalInput")   # [p-1, i, o]
    c0o = nc.dram_tensor("c0o", [OS, I], F32, kind="ExternalInput")    # [o, i]
    biasc = nc.dram_tensor("biasc", [OS, 1], F32, kind="ExternalInput")
    yt = nc.dram_tensor("yt", [OS, BS], F32, kind="ExternalOutput")    # [o, b]

    with tile.TileContext(nc) as tc, ExitStack() as ctx:
        cons = ctx.enter_context(tc.tile_pool(name="cons", bufs=1))
        c0pool = ctx.enter_context(tc.tile_pool(name="c0", bufs=4))
        cpool = ctx.enter_context(tc.tile_pool(name="coef", bufs=8))
        ppool = ctx.enter_context(tc.tile_pool(name="pow", bufs=1))
        opool = ctx.enter_context(tc.tile_pool(name="out", bufs=3))
        pspool = ctx.enter_context(
            tc.tile_pool(name="ps", bufs=8, space=bass.MemorySpace.PSUM)
        )

        # biascol[o-part, ot] = bias[o] + sum_i C0[i, o], DVE-only:
        # C0 arrives in [o, i] layout so the i-reduction is a free-dim reduce.
        red = cons.tile([128, NT], F32)
        for ot in range(NT):
            c0s = c0pool.tile([128, I], F32, tag="c0")
            nc.sync.dma_start(c0s[:], c0o[ot * 128:(ot + 1) * 128, :])
            nc.vector.tensor_reduce(
                red[:, ot:ot + 1], c0s[:], mybir.AxisListType.X, mybir.AluOpType.add
            )
        biasc_sb = cons.tile([128, NT], F32)
        for ot in range(NT):
            nc.sync.dma_start(
                biasc_sb[:, ot:ot + 1], biasc[ot * 128:(ot + 1) * 128, :]
            )
        biascol = cons.tile([128, NT], F32)
        nc.vector.tensor_add(biascol[:], red[:], biasc_sb[:])

        # 8 concurrent accumulation groups: (o-tile, b-half) -> one PSUM bank
        ps = {}
        for ot in range(NT):
            for h in range(NH):
                ps[(ot, h)] = pspool.tile(
                    [128, 512], F32, tag="ps", name=f"ps_{ot}_{h}"
                )

        for k in range(NK):
            # resident power slabs [i=128, b=BS] for this k
            x1 = ppool.tile([128, BS], F32R, tag=f"p1_{k}")
            nc.sync.dma_start(x1[:], xt[k * 128:(k + 1) * 128, :].bitcast(F32R))
            p2 = ppool.tile([128, BS], F32R, tag=f"p2_{k}")
            p3 = ppool.tile([128, BS], F32R, tag=f"p3_{k}")
            p4 = ppool.tile([128, BS], F32R, tag=f"p4_{k}")
            nc.scalar.square(p2[:], x1[:])
            nc.vector.tensor_mul(p3[:], p2[:], x1[:])
            nc.vector.tensor_mul(p4[:], p2[:], p2[:])
            pow_ = {1: x1, 2: p2, 3: p3, 4: p4}

            for p in range(1, 5):
                cpt = cpool.tile([128, OS], F32R, tag="cp")
                nc.sync.dma_start(
                    cpt[:], ct[p - 1, k * 128:(k + 1) * 128, :].bitcast(F32R)
                )
                for ot in range(NT):
                    for h in range(NH):
                        nc.tensor.matmul(
                            ps[(ot, h)],
                            cpt[:, ot * 128:(ot + 1) * 128],
                            pow_[p][:, h * 512:(h + 1) * 512],
                            start=(k == 0 and p == 1),
                            stop=(k == NK - 1 and p == 4),
                        )

        for ot in range(NT):
            for h in range(NH):
                o_sb = opool.tile([128, 512], F32)
                nc.vector.tensor_scalar_add(
                    o_sb[:], ps[(ot, h)], biascol[:, ot:ot + 1]
                )
                nc.sync.dma_start(
                    yt[ot * 128:(ot + 1) * 128, h * 512:(h + 1) * 512], o_sb[:]
                )

    nc.compile()
    return nc


def _get_nc():
    if "nc" not in _CACHE:
        _CACHE["nc"] = _build()
    return _CACHE["nc"]


def _make_in_maps(x, coeffs, bias):
    x = np.asarray(x, dtype=np.float32)
    coeffs = np.asarray(coeffs, dtype=np.float32)
    bias = np.asarray(bias, dtype=np.float32)

    xts = [
        np.ascontiguousarray(x[bg * BS:(bg + 1) * BS, :].T) for bg in range(BW)
    ]
    cts = [
        np.ascontiguousarray(
            coeffs[og * OS:(og + 1) * OS, :, 1:].transpose(2, 1, 0)
        )
        for og in range(OW)
    ]
    c0os = [
        np.ascontiguousarray(coeffs[og * OS:(og + 1) * OS, :, 0])
        for og in range(OW)
    ]
    in_maps = []
    for c in range(BW * OW):
        bg, og = c // OW, c % OW
        in_maps.append(
            {
                "xt": xts[bg],
                "ct": cts[og],
                "c0o": c0os[og],
                "biasc": np.ascontiguousarray(
                    bias[0, og * OS:(og + 1) * OS].reshape(OS, 1)
                ),
            }
        )
    return in_maps


def _gather(results):
    y = np.empty((B, O), dtype=np.float32)
    for c, res in enumerate(results):
        bg, og = c // OW, c % OW
        y[bg * BS:(bg + 1) * BS, og * OS:(og + 1) * OS] = res["yt"].T
    return y


def run(x, coeffs, bias, trace=False, **trace_kwargs):
    nc = _get_nc()
    in_maps = _make_in_maps(x, coeffs, bias)
    br = run_bass_kernel_spmd(
        nc, in_maps, list(range(BW * OW)), trace=trace, **trace_kwargs
    )
    return _gather(br.results), br


def kernel(x, coeffs, bias):
    out, _ = run(x, coeffs, bias)
    return out


# revision 7
# speedup vs baseline: 1.3979x; 1.1680x over previous
"""Trainium2 Bass kernel for KANPolyLayer:
    y[b,o] = sum_{i,p} x[b,i]^p * coeffs[o,i,p] + bias[o],  p = 0..4

Math: y = sum_{p=1..4} (x^p) @ C_p^T + (bias + colsum(C_0)), with
C_p = coeffs[:, :, p].  Implemented as 4 accumulated GEMM planes in
float32r (FP22 truncated fp32, full PE rate) with powers computed
on-chip (ScalarE square + VectorE muls).

Per-core schedule: the x^p power slabs ([i, b] layout) are resident in
SBUF; coefficient tiles stream through a small ring.  All 8 output
groups (4 o-tiles x 2 b-halves) accumulate concurrently in 8 PSUM
banks, so each arriving coefficient tile immediately unlocks 8 matmuls
and the PE never waits on the 10 MB coefficient stream.  The p=0
constant column and bias are reduced on-device with small matmuls into
a PSUM column, then applied as a per-partition scalar during the
PSUM->SBUF copy.  The kernel computes yT = [o, b]; host transposes.

Sharding (8 cores): 4 batch groups x 2 out-dim groups.
  core c -> (bg, og) = (c // 2, c % 2)
  per-core x slice:    rows [bg*1024, (bg+1)*1024)   (transposed on host)
  per-core out slice:  cols [og*512, (og+1)*512)
Each core computes a disjoint (512 x 1024) block of yT; host gathers.
"""

from contextlib import ExitStack

import numpy as np

import concourse.bacc as bacc
import concourse.bass as bass
import concourse.mybir as mybir
import concourse.tile as tile
from concourse.bass_utils import run_bass_kernel_spmd

F32 = mybir.dt.float32
F32R = mybir.dt.float32r

B, I, O = 4096, 1024, 1024  # batch, in_dim, out_dim
BW, OW = 4, 2               # batch groups x out-dim groups (8 cores)
BS, OS = B // BW, O // OW   # per-core batch (1024) and out (512)
NK = I // 128               # contraction tiles (8)
NT = OS // 128              # o-tiles (4)
NH = BS // 512              # b-halves (2)

_CACHE: dict = {}


def _build():
    nc = bacc.Bacc("TRN2", target_bir_lowering=False, debug=False, num_devices=8)

    xt = nc.dram_tensor("xt", [I, BS], F32, kind="ExternalInput")      # [i, b]
    ct = nc.dram_tensor("ct", [4, I, OS], F32, kind="ExternalInput")   # [p-1, i, o]
    c0o = nc.dram_tensor("c0o", [OS, I], F32, kind="ExternalInput")    # [o, i]
    biasc = nc.dram_tensor("biasc", [OS, 1], F32, kind="ExternalInput")
    yt = nc.dram_tensor("yt", [OS, BS], F32, kind="ExternalOutput")    # [o, b]

    with tile.TileContext(nc) as tc, ExitStack() as ctx:
        cons = ctx.enter_context(tc.tile_pool(name="cons", bufs=1))
        c0pool = ctx.enter_context(tc.tile_pool(name="c0", bufs=4))
        cpool = ctx.enter_context(tc.tile_pool(name="coef", bufs=8))
        ppool = ctx.enter_context(tc.tile_pool(name="pow", bufs=1))
        opool = ctx.enter_context(tc.tile_pool(name="out", bufs=3))
        pspool = ctx.enter_context(
            tc.tile_pool(name="ps", bufs=8, space=bass.MemorySpace.PSUM)
        )

        # 8 concurrent accumulation groups: (o-tile, b-half) -> one PSUM bank
        ps = {}
        for ot in range(NT):
            for h in range(NH):
                ps[(ot, h)] = pspool.tile(
                    [128, 512], F32, tag="ps", name=f"ps_{ot}_{h}"
                )

        for k in range(NK):
            # resident power slabs [i=128, b=BS] for this k
            x1 = ppool.tile([128, BS], F32R, tag=f"p1_{k}")
            nc.sync.dma_start(x1[:], xt[k * 128:(k + 1) * 128, :].bitcast(F32R))
            p2 = ppool.tile([128, BS], F32R, tag=f"p2_{k}")
            p3 = ppool.tile([128, BS], F32R, tag=f"p3_{k}")
            p4 = ppool.tile([128, BS], F32R, tag=f"p4_{k}")
            nc.scalar.square(p2[:], x1[:])
            nc.vector.tensor_mul(p3[:], p2[:], x1[:])
            nc.vector.tensor_mul(p4[:], p2[:], p2[:])
            pow_ = {1: x1, 2: p2, 3: p3, 4: p4}

            if k < NK - 1:
                for p in range(1, 5):
                    cpt = cpool.tile([128, OS], F32R, tag="cp")
                    nc.sync.dma_start(
                        cpt[:], ct[p - 1, k * 128:(k + 1) * 128, :].bitcast(F32R)
                    )
                    for ot in range(NT):
                        for h in range(NH):
                            nc.tensor.matmul(
                                ps[(ot, h)],
                                cpt[:, ot * 128:(ot + 1) * 128],
                                pow_[p][:, h * 512:(h + 1) * 512],
                                start=(k == 0 and p == 1),
                                stop=False,
                            )
            else:
                # last k-plane: group-contiguous so groups finish staggered
                # and the bias-add + output DMA overlap the matmul stream
                cpts = {}
                for p in range(1, 5):
                    cpt = cpool.tile([128, OS], F32R, tag="cp", name=f"cpt_l{p}")
                    nc.sync.dma_start(
                        cpt[:], ct[p - 1, k * 128:(k + 1) * 128, :].bitcast(F32R)
                    )
                    cpts[p] = cpt

        # bias/C0 inputs stream behind the main inputs (only needed at the end):
        # biascol[o-part, ot] = bias[o] + sum_i C0[i, o], DVE-only.
        red = cons.tile([128, NT], F32)
        for ot in range(NT):
            c0s = c0pool.tile([128, I], F32, tag="c0")
            nc.sync.dma_start(c0s[:], c0o[ot * 128:(ot + 1) * 128, :])
            nc.vector.tensor_reduce(
                red[:, ot:ot + 1], c0s[:], mybir.AxisListType.X, mybir.AluOpType.add
            )
        biasc_sb = cons.tile([128, NT], F32)
        for ot in range(NT):
            nc.sync.dma_start(
                biasc_sb[:, ot:ot + 1], biasc[ot * 128:(ot + 1) * 128, :]
            )
        biascol = cons.tile([128, NT], F32)
        nc.vector.tensor_add(biascol[:], red[:], biasc_sb[:])

        k = NK - 1
        for ot in range(NT):
            for h in range(NH):
                for p in range(1, 5):
                    nc.tensor.matmul(
                        ps[(ot, h)],
                        cpts[p][:, ot * 128:(ot + 1) * 128],
                        pow_[p][:, h * 512:(h + 1) * 512],
                        start=False,
                        stop=(p == 4),
                    )
                o_sb = opool.tile([128, 512], F32, tag="o_sb", name=f"o_{ot}_{h}")
                nc.vector.tensor_scalar_add(
                    o_sb[:], ps[(ot, h)], biascol[:, ot:ot + 1]
                )
                nc.sync.dma_start(
                    yt[ot * 128:(ot + 1) * 128, h * 512:(h + 1) * 512], o_sb[:]
                )

    nc.compile()
    return nc


def _get_nc():
    if "nc" not in _CACHE:
        _CACHE["nc"] = _build()
    return _CACHE["nc"]


def _make_in_maps(x, coeffs, bias):
    x = np.asarray(x, dtype=np.float32)
    coeffs = np.asarray(coeffs, dtype=np.float32)
    bias = np.asarray(bias, dtype=np.float32)

    xts = [
        np.ascontiguousarray(x[bg * BS:(bg + 1) * BS, :].T) for bg in range(BW)
    ]
    cts = [
        np.ascontiguousarray(
            coeffs[og * OS:(og + 1) * OS, :, 1:].transpose(2, 1, 0)
        )
        for og in range(OW)
    ]
    c0os = [
        np.ascontiguousarray(coeffs[og * OS:(og + 1) * OS, :, 0])
        for og in range(OW)
    ]
    in_maps = []
    for c in range(BW * OW):
        bg, og = c // OW, c % OW
        in_maps.append(
            {
                "xt": xts[bg],
                "ct": cts[og],
                "c0o": c0os[og],
                "biasc": np.ascontiguousarray(
                    bias[0, og * OS:(og + 1) * OS].reshape(OS, 1)
                ),
            }
        )
    return in_maps


def _gather(results):
    y = np.empty((B, O), dtype=np.float32)
    for c, res in enumerate(results):
        bg, og = c // OW, c % OW
        y[bg * BS:(bg + 1) * BS, og * OS:(og + 1) * OS] = res["yt"].T
    return y


def run(x, coeffs, bias, trace=False, **trace_kwargs):
    nc = _get_nc()
    in_maps = _make_in_maps(x, coeffs, bias)
    br = run_bass_kernel_spmd(
        nc, in_maps, list(range(BW * OW)), trace=trace, **trace_kwargs
    )
    return _gather(br.results), br


def kernel(x, coeffs, bias):
    out, _ = run(x, coeffs, bias)
    return out


# revision 8
# speedup vs baseline: 1.4322x; 1.0246x over previous
"""Trainium2 Bass kernel for KANPolyLayer:
    y[b,o] = sum_{i,p} x[b,i]^p * coeffs[o,i,p] + bias[o],  p = 0..4

Math: y = sum_{p=1..4} (x^p) @ C_p^T + (bias + colsum(C_0)), with
C_p = coeffs[:, :, p].  Implemented as 4 accumulated GEMM planes in
float32r (FP22 truncated fp32, full PE rate) with powers computed
on-chip (ScalarE square + VectorE muls).

Per-core schedule: the x^p power slabs ([i, b] layout) are resident in
SBUF; coefficient tiles stream through a small ring.  All 8 output
groups (4 o-tiles x 2 b-halves) accumulate concurrently in 8 PSUM
banks, so each arriving coefficient tile immediately unlocks 8 matmuls
and the PE never waits on the 10 MB coefficient stream.  The p=0
constant column and bias are reduced on-device with small matmuls into
a PSUM column, then applied as a per-partition scalar during the
PSUM->SBUF copy.  The kernel computes yT = [o, b]; host transposes.

Sharding (8 cores): 4 batch groups x 2 out-dim groups.
  core c -> (bg, og) = (c // 2, c % 2)
  per-core x slice:    rows [bg*1024, (bg+1)*1024)   (transposed on host)
  per-core out slice:  cols [og*512, (og+1)*512)
Each core computes a disjoint (512 x 1024) block of yT; host gathers.
"""

from contextlib import ExitStack

import numpy as np

import concourse.bacc as bacc
import concourse.bass as bass
import concourse.mybir as mybir
import concourse.tile as tile
from concourse.bass_utils import run_bass_kernel_spmd

F32 = mybir.dt.float32
F32R = mybir.dt.float32r

B, I, O = 4096, 1024, 1024  # batch, in_dim, out_dim
BW, OW = 4, 2               # batch groups x out-dim groups (8 cores)
BS, OS = B // BW, O // OW   # per-core batch (1024) and out (512)
NK = I // 128               # contraction tiles (8)
NT = OS // 128              # o-tiles (4)
NH = BS // 512              # b-halves (2)

_CACHE: dict = {}


def _build():
    nc = bacc.Bacc("TRN2", target_bir_lowering=False, debug=False, num_devices=8)

    xt = nc.dram_tensor("xt", [I, BS], F32, kind="ExternalInput")      # [i, b]
    ct = nc.dram_tensor("ct", [4, I, OS], F32, kind="ExternalInput")   # [p-1, i, o]
    c0o = nc.dram_tensor("c0o", [OS, I], F32, kind="ExternalInput")    # [o, i]
    biasc = nc.dram_tensor("biasc", [OS, 1], F32, kind="ExternalInput")
    yt = nc.dram_tensor("yt", [OS, BS], F32, kind="ExternalOutput")    # [o, b]

    NTAIL = 2  # trailing k-planes emitted group-contiguous (tail stagger)

    with tile.TileContext(nc) as tc, ExitStack() as ctx:
        cons = ctx.enter_context(tc.tile_pool(name="cons", bufs=1))
        c0pool = ctx.enter_context(tc.tile_pool(name="c0", bufs=4))
        cpool = ctx.enter_context(tc.tile_pool(name="coef", bufs=12))
        ppool = ctx.enter_context(tc.tile_pool(name="pow", bufs=1))
        opool = ctx.enter_context(tc.tile_pool(name="out", bufs=3))
        pspool = ctx.enter_context(
            tc.tile_pool(name="ps", bufs=8, space=bass.MemorySpace.PSUM)
        )

        # 8 concurrent accumulation groups: (o-tile, b-half) -> one PSUM bank
        ps = {}
        for ot in range(NT):
            for h in range(NH):
                ps[(ot, h)] = pspool.tile(
                    [128, 512], F32, tag="ps", name=f"ps_{ot}_{h}"
                )

        # PE warmup: garbage matmuls on a memset tile while the first input
        # DMAs are in flight, so the HAM clock-gate reaches 2.4 GHz before
        # the real stream starts (saves the ~2us cold-start penalty).
        wz = cons.tile([128, 512], F32)
        nc.vector.memset(wz[:], 0.0)
        wr = cons.tile([128, 512], F32R)
        nc.vector.tensor_copy(wr[:], wz[:])
        for w in range(9):
            nc.tensor.matmul(
                ps[(0, 0)], wr[:, 0:128], wr[:], start=True, stop=True,
                skip_group_check=True,
            )

        pows = {}
        cpts = {}
        for k in range(NK):
            tail_k = k >= NK - NTAIL
            # k0: coefficient tile first (smaller -> lands first)
            if k == 0:
                cpt = cpool.tile([128, OS], F32R, tag="cp", name="cpt_0_1")
                nc.sync.dma_start(cpt[:], ct[0, 0:128, :].bitcast(F32R))
                cpts[(0, 1)] = cpt
            # resident power slabs [i=128, b=BS] for this k
            x1 = ppool.tile([128, BS], F32R, tag=f"p1_{k}", name=f"x1_{k}")
            nc.sync.dma_start(x1[:], xt[k * 128:(k + 1) * 128, :].bitcast(F32R))
            p2 = ppool.tile([128, BS], F32R, tag=f"p2_{k}", name=f"p2_{k}")
            p3 = ppool.tile([128, BS], F32R, tag=f"p3_{k}", name=f"p3_{k}")
            p4 = ppool.tile([128, BS], F32R, tag=f"p4_{k}", name=f"p4_{k}")
            nc.scalar.square(p2[:], x1[:])
            nc.vector.tensor_mul(p3[:], p2[:], x1[:])
            nc.vector.tensor_mul(p4[:], p2[:], p2[:])
            pows[k] = {1: x1, 2: p2, 3: p3, 4: p4}

            for p in range(1, 5):
                if (k, p) not in cpts:
                    cpt = cpool.tile(
                        [128, OS], F32R, tag="cp", name=f"cpt_{k}_{p}"
                    )
                    nc.sync.dma_start(
                        cpt[:], ct[p - 1, k * 128:(k + 1) * 128, :].bitcast(F32R)
                    )
                    cpts[(k, p)] = cpt
                if not tail_k:
                    for ot in range(NT):
                        for h in range(NH):
                            nc.tensor.matmul(
                                ps[(ot, h)],
                                cpts[(k, p)][:, ot * 128:(ot + 1) * 128],
                                pows[k][p][:, h * 512:(h + 1) * 512],
                                start=(k == 0 and p == 1),
                                stop=False,
                            )

        # bias/C0 inputs stream behind the main inputs (only needed at end):
        # biascol[o-part, ot] = bias[o] + sum_i C0[i, o], DVE-only.
        red = cons.tile([128, NT], F32)
        for ot in range(NT):
            c0s = c0pool.tile([128, I], F32, tag="c0", name=f"c0s_{ot}")
            nc.sync.dma_start(c0s[:], c0o[ot * 128:(ot + 1) * 128, :])
            nc.vector.tensor_reduce(
                red[:, ot:ot + 1], c0s[:], mybir.AxisListType.X, mybir.AluOpType.add
            )
        biasc_sb = cons.tile([128, NT], F32)
        for ot in range(NT):
            nc.sync.dma_start(
                biasc_sb[:, ot:ot + 1], biasc[ot * 128:(ot + 1) * 128, :]
            )
        biascol = cons.tile([128, NT], F32)
        nc.vector.tensor_add(biascol[:], red[:], biasc_sb[:])

        # trailing k-planes group-contiguous: each group finishes ~2.1us
        # apart, so bias-add + output DMA overlap the matmul stream
        for ot in range(NT):
            for h in range(NH):
                for k in range(NK - NTAIL, NK):
                    for p in range(1, 5):
                        nc.tensor.matmul(
                            ps[(ot, h)],
                            cpts[(k, p)][:, ot * 128:(ot + 1) * 128],
                            pows[k][p][:, h * 512:(h + 1) * 512],
                            start=False,
                            stop=(k == NK - 1 and p == 4),
                        )
                o_sb = opool.tile([128, 512], F32, tag="o_sb", name=f"o_{ot}_{h}")
                if (ot + h) % 2 == 0:
                    nc.vector.tensor_scalar_add(
                        o_sb[:], ps[(ot, h)], biascol[:, ot:ot + 1]
                    )
                else:
                    nc.scalar.activation(
                        o_sb[:],
                        ps[(ot, h)],
                        mybir.ActivationFunctionType.Identity,
                        bias=biascol[:, ot:ot + 1],
                    )
                nc.sync.dma_start(
                    yt[ot * 128:(ot + 1) * 128, h * 512:(h + 1) * 512], o_sb[:]
                )

    nc.compile()
    return nc


def _get_nc():
    if "nc" not in _CACHE:
        _CACHE["nc"] = _build()
    return _CACHE["nc"]


def _make_in_maps(x, coeffs, bias):
    x = np.asarray(x, dtype=np.float32)
    coeffs = np.asarray(coeffs, dtype=np.float32)
    bias = np.asarray(bias, dtype=np.float32)

    xts = [
        np.ascontiguousarray(x[bg * BS:(bg + 1) * BS, :].T) for bg in range(BW)
    ]
    cts = [
        np.ascontiguousarray(
            coeffs[og * OS:(og + 1) * OS, :, 1:].transpose(2, 1, 0)
        )
        for og in range(OW)
    ]
    c0os = [
        np.ascontiguousarray(coeffs[og * OS:(og + 1) * OS, :, 0])
        for og in range(OW)
    ]
    in_maps = []
    for c in range(BW * OW):
        bg, og = c // OW, c % OW
        in_maps.append(
            {
                "xt": xts[bg],
                "ct": cts[og],
                "c0o": c0os[og],
                "biasc": np.ascontiguousarray(
                    bias[0, og * OS:(og + 1) * OS].reshape(OS, 1)
                ),
            }
        )
    return in_maps


def _gather(results):
    y = np.empty((B, O), dtype=np.float32)
    for c, res in enumerate(results):
        bg, og = c // OW, c % OW
        y[bg * BS:(bg + 1) * BS, og * OS:(og + 1) * OS] = res["yt"].T
    return y


def run(x, coeffs, bias, trace=False, **trace_kwargs):
    nc = _get_nc()
    in_maps = _make_in_maps(x, coeffs, bias)
    br = run_bass_kernel_spmd(
        nc, in_maps, list(range(BW * OW)), trace=trace, **trace_kwargs
    )
    return _gather(br.results), br


def kernel(x, coeffs, bias):
    out, _ = run(x, coeffs, bias)
    return out
